# revision 25
# baseline (speedup 1.0000x reference)
"""Trainium2 Bass kernel for nn_Attention_39384850104955 (single-launch).

Dense multi-head attention (B=2, S=2048, D=1024, H=16, dh=64) with a
materialized [B,H,S,S] score tensor plus additive bias, eager softmax,
and in/out projections.

Sharding: head-parallel across 8 NeuronCores; core c owns heads
{2c, 2c+1} for BOTH batches, so each bias head is read exactly once
across the fleet.

v4 design: the additive bias is folded in multiplicatively —
exp(s + b) = exp(s) * exp(b) — with exp(bias) precomputed on the host,
so v1's per-tile PE bias-injection matmuls become wide DVE multiplies
after the ScalarE exp. QKV evacuations ride the otherwise-idle
ScalarE. The attention loop runs 4 outer iterations of sq=512. The
cross-core reduction for the output projection is exchanged in
FACTORED form: instead of ReduceScatter-ing 8 MiB of [B*S, D] partial
projections, cores AllToAll the per-head attention outputs (oc) so
each core gathers all 1024 head-dims for its own 128 rows per
iteration (1 MiB of traffic total, 8x less), then applies the full
output projection locally. Each iteration's AllToAll flies behind the
next iteration's attention; its output projection is emitted after
that attention so only the last exchange is latency-exposed.
"""

import sys

sys.path.insert(0, "/opt/trn_rl_repo")

import numpy as np
import jax.numpy as jnp

import concourse.bacc as bacc
import concourse.mybir as mybir
import concourse.tile as tile
from concourse.bass_utils import run_bass_kernel_spmd

f32 = mybir.dt.float32
f16 = mybir.dt.float16
f32r = mybir.dt.float32r

P = 128
B, S, D, H, DH = 2, 2048, 1024, 16, 64
NCORE = 8
NT = S // P           # 16 sk tiles per batch
SQB = 512             # sq span per outer iteration
NQB = S // SQB        # 2 outer iterations
SBLK = 1024           # projection seq block
NBLK = B * S // SBLK  # 4 projection blocks
RW = B * SQB // NCORE  # 128 output rows per core per iteration
SCALE = 1.0 / 8.0     # 1/sqrt(dh)

Exp = mybir.ActivationFunctionType.Exp
Copy = mybir.ActivationFunctionType.Copy
Mult = mybir.AluOpType.mult

_CACHE = {}


def _emit_body(nc, tc, w_s, woc, ident16, xT, ebT, cins, couts, fin,
               qk_pool, apool, skip=()):
    hsl = [slice(0, 64), slice(64, 128)]
    qT_s = qk_pool.tile([P, B * S], f16, tag="qT", name="qT")  # pre-scaled
    kT_s = qk_pool.tile([P, B * S], f16, tag="kT", name="kT")
    # v_s[:, t, :]: cols 0:64 = head A dims, 64 = ones (head A row-sum),
    # 96:160 = head B dims, 160 = ones (head B row-sum)
    v_s = qk_pool.tile([P, 2 * NT, 192], f16, tag="v", name="v")
    nc.vector.memset(v_s[:, :, 64:65], 1.0)
    nc.vector.memset(v_s[:, :, 160:161], 1.0)

    # ---------------- QKV projections ----------------
    # Unit = (proj w, block); k and v first so attention can start before
    # the q tail; q blocks ordered 0,2,1,3 to unblock sq-iter 0 first.
    # Each unit's [128, 1024] PSUM output rotates through the shared
    # "psg" tag (same tag the attention phase uses - no pool barrier).
    with tc.tile_pool(name="xload", bufs=1) as xload, \
         tc.tile_pool(name="vstage", bufs=2) as vstage:
        # k first (scores need all of kT), then the q blocks for sq-iters
        # 0-1 so attention's exp stream starts after 6 of 12 units; v and
        # the late q blocks overlap attention (PE has slack there).
        # QKV PSUM rides the pa0/pa1 tag slots, which attention only
        # needs once its first attn@V chain starts - so attention's
        # psg-tag scores don't wait behind the QKV rotation.
        units = [(1, 0), (1, 1), (1, 2), (1, 3),
                 (0, 0), (0, 2),
                 (2, 0), (2, 1), (2, 2), (2, 3),
                 (0, 1), (0, 3)]
        nch = 1 if "proj" in skip else 8
        pslot = 0
        xts = {}
        for ui, (w, blk) in enumerate(units):
            if blk not in xts:
                xt = xts[blk] = xload.tile([P, 8, SBLK], f16,
                                           tag=f"xt{blk}", name=f"xt{blk}")
                if "xdma" not in skip:
                    if ui == 0:
                        # chunked first load: first matmul starts when
                        # chunk 0 lands instead of after the whole 2 MiB
                        for c in range(8):
                            nc.sync.dma_start(
                                xt[:, c, :],
                                xT[c * P:(c + 1) * P,
                                   blk * SBLK:(blk + 1) * SBLK])
                    else:
                        nc.sync.dma_start(
                            xt[:],
                            xT[:, blk * SBLK:(blk + 1) * SBLK]
                            .rearrange("(c p) n -> p c n", p=P))
                else:
                    nc.vector.memset(xt[0:1, 0, 0:1], 0.0)
            xt = xts[blk]
            ppt = apool.tile([P, 2, 512], f32, tag=f"pa{pslot}",
                             name="ppt", bufs=1)
            pslot = 1 - pslot
            # hf-outer: each PSUM bank's accumulation group closes before
            # the next opens (interleaved open groups corrupt results)
            for hf in range(2):
                for c in range(nch):
                    nc.tensor.matmul(
                        ppt[:, hf, :], w_s[:, w, c, :],
                        xt[:, c, hf * 512:(hf + 1) * 512],
                        start=(c == 0), stop=(c == nch - 1))
            cols = slice(blk * SBLK, (blk + 1) * SBLK)
            if w == 0:
                # late q evacs ride DVE - ScalarE is the attention
                # bottleneck once the exp stream starts
                if ui < 6:
                    nc.scalar.activation(qT_s[:, cols], ppt[:], Copy,
                                         scale=SCALE)
                else:
                    nc.vector.tensor_scalar_mul(qT_s[:, cols], ppt[:],
                                                SCALE)
            elif w == 1:
                nc.scalar.activation(kT_s[:, cols], ppt[:], Copy)
            else:
                vst = vstage.tile([P, SBLK], f16, tag="vst", name="vst")
                nc.vector.tensor_copy(vst[:], ppt[:])
                # PE-transpose the 8 [128,128] chunks into one f16 PSUM
                # tile, then two wide strided copies into v_s
                pvt = apool.tile([P, 8, P], f16, tag=f"pa{pslot}",
                                 name="pvt", bufs=1)
                pslot = 1 - pslot
                for a in range(8):
                    nc.tensor.matmul(pvt[:, a, :],
                                     vst[:, a * P:(a + 1) * P],
                                     ident16[:], is_transpose=True,
                                     start=True, stop=True)
                nc.vector.tensor_copy(
                    v_s[:, blk * 8:(blk + 1) * 8, 0:64], pvt[:, :, 0:64])
                nc.vector.tensor_copy(
                    v_s[:, blk * 8:(blk + 1) * 8, 96:160],
                    pvt[:, :, 64:128])

    # ---------------- attention ----------------
    with tc.tile_pool(name="slabp", bufs=2) as slab_pool, \
         tc.tile_pool(name="expp", bufs=2) as exp_pool, \
         tc.tile_pool(name="nrm", bufs=2) as nrm_pool, \
         tc.tile_pool(name="ocp", bufs=2) as oc_pool, \
         tc.tile_pool(name="ogp", bufs=2) as og_pool, \
         tc.tile_pool(name="ptp", bufs=2) as pt_pool:

        woc_s = og_pool.tile([P, NCORE, D], f16, tag="woc",
                             name="woc_s", bufs=1)
        woc_loaded = []

        def _oproj(it):
            # local full-D output projection for this core's RW rows
            if not woc_loaded:
                nc.sync.dma_start(woc_s[:], woc)
                woc_loaded.append(True)
            ocg = og_pool.tile([P, NCORE, RW], f16, tag="ocg", name="ocg")
            src = couts[it] if "rs" not in skip else cins[it]
            nc.sync.dma_start(ocg[:], src.rearrange("q p n -> p q n"))
            pf = apool.tile([P, 2, 512], f32, tag="psg", name="pf",
                            bufs=2)
            nq = 1 if "oproj" in skip else NCORE
            for nh in range(2):
                for q in range(nq):
                    nc.tensor.matmul(
                        pf[:, nh, :], ocg[:, q, :],
                        woc_s[:, q, nh * 512:(nh + 1) * 512],
                        start=(q == 0), stop=(q == nq - 1))
            pt = pt_pool.tile([P, 2, 512], f16, tag="pt", name="pt")
            nc.vector.tensor_copy(pt[:], pf[:])
            nc.sync.dma_start(fin[it], pt[:])

        for it in range(NQB):
            sq0 = it * SQB
            oc = {bb: oc_pool.tile([P, SQB], f16, tag=f"oc{bb}",
                                   name=f"oc{bb}") for bb in range(2)}
            for h in range(2):
                for cc in range(1):
                    cq0 = sq0 + cc * 512
                    slab = slab_pool.tile([P, NT, 512], f16, tag="slab",
                                          name="slab")
                    if "bdma" not in skip:
                        nc.sync.dma_start(
                            slab[:],
                            ebT[h, :, cq0:cq0 + 512]
                            .rearrange("(t p) n -> p t n", p=P))
                    else:
                        nc.vector.memset(slab[0:1, 0, 0:1], 1.0)
                    expcs = {bb: exp_pool.tile([P, NT, 512], f16,
                                               tag=f"expc{bb}",
                                               name=f"expc{bb}")
                             for bb in range(2)}
                    for g in range(8):
                        for bb in range(2):
                            psg = apool.tile([P, 2, 512], f32, tag="psg",
                                             name="psg", bufs=2)
                            for j in range(2):
                                t = bb * NT + 2 * g + j
                                nc.tensor.matmul(
                                    psg[:, j, :],
                                    kT_s[hsl[h], t * P:(t + 1) * P],
                                    qT_s[hsl[h], bb * S + cq0:
                                         bb * S + cq0 + 512],
                                    start=True, stop=True)
                            ec = expcs[bb][:, 2 * g:2 * g + 2, :]
                            nc.scalar.activation(ec, psg[:], Exp)
                            nc.vector.tensor_tensor(
                                ec, ec, slab[:, 2 * g:2 * g + 2, :], Mult)
                    pa = {bb: apool.tile([P, 512], f32, tag=f"pa{bb}",
                                         name=f"pa{bb}", bufs=1)
                          for bb in range(2)}
                    vc0 = 0 if h == 0 else 96
                    nts = 2 if "attnv" in skip else NT
                    for bb in range(2):
                        for t in range(nts):
                            nc.tensor.matmul(
                                pa[bb][0:65, :],
                                v_s[:, bb * NT + t, vc0:vc0 + 65],
                                expcs[bb][:, t, :],
                                start=(t == 0), stop=(t == nts - 1))
                    for bb in range(2):
                        srow, vrows = pa[bb][64:65, :], pa[bb][0:64, :]
                        recip = nrm_pool.tile([1, 512], f32, tag="recip",
                                              name="recip")
                        nc.vector.reciprocal(recip[:], srow)
                        rbc = nrm_pool.tile([64, 512], f32, tag="rbc",
                                            name="rbc")
                        nc.gpsimd.partition_broadcast(rbc[:], recip[:])
                        nc.vector.tensor_tensor(
                            oc[bb][hsl[h], cc * 512:(cc + 1) * 512],
                            vrows, rbc[:], Mult)
            # factored exchange: send each peer its rows of our head-dims
            for p in range(NCORE):
                bb_p, ci_p = p // 4, p % 4
                nc.sync.dma_start(
                    cins[it][p],
                    oc[bb_p][:, ci_p * RW:(ci_p + 1) * RW])
            if "rs" not in skip:
                nc.gpsimd.collective_compute(
                    "AllToAll", mybir.AluOpType.bypass,
                    replica_groups=[list(range(NCORE))],
                    ins=[cins[it]], outs=[couts[it]])
            # project the previous iteration's rows while this iteration's
            # exchange flies behind the next iteration's attention
            if it > 0:
                _oproj(it - 1)
        _oproj(NQB - 1)


def build_full(repeat=1, skip=()):
    nc = bacc.Bacc("TRN2", target_bir_lowering=False, debug=False,
                   num_devices=NCORE)
    xT = nc.dram_tensor("xT", [D, B * S], f16, kind="ExternalInput").ap()
    wT = nc.dram_tensor("wT", [3, D, P], f16, kind="ExternalInput").ap()
    ebT = nc.dram_tensor("ebT", [2, S, S], f16, kind="ExternalInput").ap()
    woc = nc.dram_tensor("woc", [P, NCORE, D], f16,
                         kind="ExternalInput").ap()
    identr = nc.dram_tensor("identr", [P, P], f32r,
                            kind="ExternalInput").ap()
    fin = nc.dram_tensor("fin", [NQB, RW, D], f16,
                         kind="ExternalOutput").ap()
    cins = [nc.dram_tensor(f"cin{q}", [NCORE, P, RW], f16).ap()
            for q in range(NQB)]
    couts = [nc.dram_tensor(f"cout{q}", [NCORE, P, RW], f16).ap()
             for q in range(NQB)]

    with tile.TileContext(nc) as tc:
        with tc.tile_pool(name="const", bufs=1) as const_pool, \
             tc.tile_pool(name="qk", bufs=1) as qk_pool, \
             tc.tile_pool(name="apsum", bufs=1, space="PSUM") as apool:
            ident_s = const_pool.tile([P, P], f32r, tag="ident",
                                      name="ident")
            nc.sync.dma_start(ident_s[:], identr)
            ident16 = const_pool.tile([P, P], f16, tag="ident16",
                                      name="ident16")
            nc.vector.tensor_copy(ident16[:], ident_s[:].bitcast(f32))
            w_s = const_pool.tile([P, 3, 8, P], f16, tag="w", name="w")
            nc.sync.dma_start(w_s[:], wT.rearrange("w (c p) m -> p w c m",
                                                   p=P))
            for _rep in range(repeat):
                _emit_body(nc, tc, w_s, woc, ident16, xT, ebT, cins,
                           couts, fin, qk_pool, apool, skip=skip)

    nc.compile()
    return nc


def _get(name, builder):
    if name not in _CACHE:
        _CACHE[name] = builder()
    return _CACHE[name]


def make_in_maps(hidden_states, bias, Wq, Wk, Wv, Wo):
    xT = np.asarray(jnp.asarray(hidden_states.reshape(B * S, D).T,
                                dtype=jnp.float16))
    # exp(bias), transposed per head to [H, sk, sq] (host-side prep)
    ebT16 = np.asarray(jnp.asarray(
        jnp.exp(jnp.asarray(bias[0])).transpose(0, 2, 1),
        dtype=jnp.float16))
    # full Wo^T arranged [dim-within-chunk, chunk, D] (replicated)
    wocF = np.asarray(jnp.asarray(Wo.T.reshape(NCORE, P, D)
                                  .transpose(1, 0, 2), dtype=jnp.float16))
    ident = np.eye(P, dtype=np.float32)
    in_maps = []
    for c in range(NCORE):
        r0 = c * P
        wTc = np.stack([np.asarray(jnp.asarray(W[r0:r0 + P, :].T,
                                               dtype=jnp.float16))
                        for W in (Wq, Wk, Wv)])
        in_maps.append({
            "xT": xT,
            "wT": wTc,
            "ebT": ebT16[2 * c:2 * c + 2],
            "woc": wocF,
            "identr": ident,
        })
    return in_maps


def assemble(results):
    out = np.empty((B * S, D), dtype=np.float32)
    for c in range(NCORE):
        finc = np.asarray(results[c]["fin"], dtype=np.float32)
        bb, ci = c // 4, c % 4
        for it in range(NQB):
            r0 = bb * S + it * SQB + ci * RW
            out[r0:r0 + RW] = finc[it]
    return out.reshape(B, S, D)


def kernel(hidden_states, bias, Wq, Wk, Wv, Wo):
    hidden_states = np.ascontiguousarray(hidden_states, dtype=np.float32)
    bias = np.ascontiguousarray(bias, dtype=np.float32)
    Wq = np.ascontiguousarray(Wq, dtype=np.float32)
    Wk = np.ascontiguousarray(Wk, dtype=np.float32)
    Wv = np.ascontiguousarray(Wv, dtype=np.float32)
    Wo = np.ascontiguousarray(Wo, dtype=np.float32)

    nc = _get("full", build_full)
    in_maps = make_in_maps(hidden_states, bias, Wq, Wk, Wv, Wo)
    res = run_bass_kernel_spmd(nc, in_maps, list(range(NCORE))).results
    return assemble(res)
